# revision 13
# baseline (speedup 1.0000x reference)
"""Fused single-head attention (QKV projection + softmax(QK^T/8) @ V) on 8
Trainium2 NeuronCores.

Problem: x [4, 2048, 1024] f32, kernel [3, 1024, 1024] f32 ->
         out [4, 2048, 1024] f32.

Sharding: 8 cores = 4 batches x 2 query-halves. Each core computes K^T and V
for its whole batch (redundantly with its pair core) plus Q for its query
half, then attention for its 1024 queries. No collectives (pair-exchange of
K/V through AllGather would cost more than the ~50us of PE it saves: ~20us
latency floor + 4MB at ~50GB/s effective).

Everything on-chip is bf16: HW-measured matmul rate is ~1 cycle/row for
bf16 (~210ns per [128x128]x[128x512]) with ldweights fully hidden, vs
~3.3 cyc/row for float32r when the stationary rotates (its weight load
serializes).  PSUM accumulation stays f32; normalization + output are f32.
End-to-end max-rel error vs the f32 reference is ~1e-2 (gate 2e-2).

x^T for the whole batch lives in SBUF (32KB/partition in bf16), so x is
DMA'd exactly once; K^T, Q^T, V, and exp(S^T) stay on-chip.  Each 512-wide
output tile is produced by one sequential 8-deep accumulation chain in a
single PSUM bank (HW-measured ~5% faster than interleaving chains across
banks), with the pool's 7-slot rotation double-buffering chains against
their PSUM->SBUF copies:

  P1 KT[o,k]  = sum_d Wk[d,o]^T x^T[d,k]
  P2 QT[o,q]  = sum_d Wq[d,o]^T x^T[d,q]
  P3 V[k,o]   = sum_d x^T[d,k]^T Wv[d,o]
  P4 E[k,q]   = exp(1/8 sum_o KT[o,k]^T QT[o,q])   (ACT writes bf16 E)
  P5 out[q,o] = (sum_k E[k,q]^T V[k,o]) / (sum_k E[k,q])
               (one stationary E[kc,qs] slice feeds both V halves and the
                ones-column denominator chain in a dedicated PSUM bank)

The transposed-scores layout needs no on-chip transposes and the softmax
needs no vector-engine reductions (denominator rides the AV stationary as a
ones-column matmul; no max-subtraction: |S/8| <~ 10 so exp stays finite).
"""

import numpy as np
from contextlib import ExitStack

import ml_dtypes

import concourse.bacc as bacc
import concourse.mybir as mybir
import concourse.tile as tile
from concourse.bass_utils import run_bass_kernel_spmd

F32 = mybir.dt.float32
BF16 = mybir.dt.bfloat16
EXP = mybir.ActivationFunctionType.Exp

B, S, D, DO = 4, 2048, 1024, 1024
QH = S // 2        # queries per core
DC = D // 128      # 8  contraction chunks
OC = DO // 128     # 8  output-dim chunks
KC = S // 128      # 16 key chunks
SCALE = 1.0 / 8.0  # 1/sqrt(64) hardcoded in the reference


def _declare_io(nc):
    xt_d = nc.dram_tensor("xt", [128, DC, S], BF16, kind="ExternalInput").ap()
    wq_d = nc.dram_tensor("wq", [128, DC, DO], BF16, kind="ExternalInput").ap()
    wk_d = nc.dram_tensor("wk", [128, DC, DO], BF16, kind="ExternalInput").ap()
    wv_d = nc.dram_tensor("wv", [128, DC, DO], BF16, kind="ExternalInput").ap()
    # out block idx = qs*2 + oh -> rows qs*128, cols oh*512
    out_d = nc.dram_tensor("out", [16, 128, 512], F32, kind="ExternalOutput").ap()
    return xt_d, wq_d, wk_d, wv_d, out_d


def _attention_core(tc, io=None, upto=5):
    nc = tc.nc
    xt_d, wq_d, wk_d, wv_d, out_d = io or _declare_io(nc)

    with ExitStack() as ctx:
        # Persistent on-chip tensors (per partition: 32+16+32 = 80KB)
        pers = ctx.enter_context(tc.tile_pool(name="pers", bufs=1))
        KT = pers.tile([128, OC, S], BF16, tag="KT")
        QT = pers.tile([128, OC, QH], BF16, tag="QT")
        V = pers.tile([128, KC, DO], BF16, tag="V")
        cpool = ctx.enter_context(tc.tile_pool(name="cpool", bufs=1))
        ones = cpool.tile([128, 2], BF16, tag="ones")
        nc.vector.memset(ones, 1.0)
        # One PSUM pool for every 512-wide accumulation chain (7 banks,
        # rotation double-buffers chains vs copies and carries no phase
        # boundaries), plus a dedicated bank for the denominator chains.
        psum = ctx.enter_context(tc.tile_pool(name="psum", bufs=7,
                                              space="PSUM"))
        dpsum = ctx.enter_context(tc.tile_pool(name="dpsum", bufs=1,
                                               space="PSUM"))

        # ---- P1-P3: projections (x^T and weights resident) -----------
        with ExitStack() as pc:
            xw = pc.enter_context(tc.tile_pool(name="xw", bufs=1))
            xt = xw.tile([128, DC, S], BF16, tag="xt")       # 32KB/part
            wk = xw.tile([128, DC, DO], BF16, tag="wk")      # 16KB/part
            wq = xw.tile([128, DC, DO], BF16, tag="wq")
            wv = xw.tile([128, DC, DO], BF16, tag="wv")
            # Startup-critical DMAs: xt fans out over the three DMA-capable
            # queues (SP, Activation, GPSIMD); wk's first o-chunk jumps
            # ahead of the bulk so P1 can start.
            nc.sync.dma_start(xt[:, 0:3], xt_d[:, 0:3])
            nc.gpsimd.dma_start(xt[:, 3:6], xt_d[:, 3:6])
            nc.scalar.dma_start(xt[:, 6:8], xt_d[:, 6:8])
            nc.scalar.dma_start(wk[:, :, 0:128], wk_d[:, :, 0:128])
            nc.scalar.dma_start(wk[:, :, 128:DO], wk_d[:, :, 128:DO])
            nc.sync.dma_start(wq, wq_d)
            nc.gpsimd.dma_start(wv, wv_d)

            # P1: KT[o-part, oc, k]
            for oc in range(OC):
                for k4 in range(4):
                    ps = psum.tile([128, 512], F32, tag="ps",
                                   name=f"psk{oc}_{k4}")
                    for dc in range(DC):
                        nc.tensor.matmul(
                            ps, wk[:, dc, oc * 128:(oc + 1) * 128],
                            xt[:, dc, k4 * 512:(k4 + 1) * 512],
                            start=(dc == 0), stop=(dc == DC - 1))
                    nc.vector.tensor_copy(
                        out=KT[:, oc, k4 * 512:(k4 + 1) * 512], in_=ps)

            if upto == 1:
                nc.sync.dma_start(out_d[0].bitcast(BF16), KT[:, 0, 0:512])
                return

            # P2: QT[o-part, oc, q].  The host rolls each odd core's key
            # axis by -QH, so xt columns [0, QH) are always this core's
            # queries.
            for oc in range(OC):
                for q2 in range(2):
                    ps = psum.tile([128, 512], F32, tag="ps",
                                   name=f"psq{oc}_{q2}")
                    for dc in range(DC):
                        nc.tensor.matmul(
                            ps, wq[:, dc, oc * 128:(oc + 1) * 128],
                            xt[:, dc, q2 * 512:(q2 + 1) * 512],
                            start=(dc == 0), stop=(dc == DC - 1))
                    nc.vector.tensor_copy(
                        out=QT[:, oc, q2 * 512:(q2 + 1) * 512], in_=ps)

            if upto == 2:
                nc.sync.dma_start(out_d[0].bitcast(BF16), QT[:, 0, 0:512])
                return

            # P3: V[k-part, kc, o].  Stationary is the x^T chunk.
            for kc in range(KC):
                for oh in range(2):
                    ps = psum.tile([128, 512], F32, tag="ps",
                                   name=f"psv{kc}_{oh}")
                    for dc in range(DC):
                        nc.tensor.matmul(
                            ps, xt[:, dc, kc * 128:(kc + 1) * 128],
                            wv[:, dc, oh * 512:(oh + 1) * 512],
                            start=(dc == 0), stop=(dc == DC - 1))
                    nc.vector.tensor_copy(
                        out=V[:, kc, oh * 512:(oh + 1) * 512], in_=ps)

        if upto == 3:
            nc.sync.dma_start(out_d[0].bitcast(BF16), V[:, 0, 0:512])
            return

        # ---- P4: E[k-part, kc, q] = exp(S^T/8) ------------------------
        # ---- P5: out[q, o] = E^T V / (E^T 1) --------------------------
        with ExitStack() as pc:
            expp = pc.enter_context(tc.tile_pool(name="expp", bufs=1))
            E = expp.tile([128, KC, QH], BF16, tag="E")      # 32KB/part
            opool = pc.enter_context(tc.tile_pool(name="opool", bufs=4))
            rpool = pc.enter_context(tc.tile_pool(name="rpool", bufs=4))

            for kc in range(KC):
                for q2 in range(2):
                    ps = psum.tile([128, 512], F32, tag="ps",
                                   name=f"pss{kc}_{q2}")
                    for oc in range(OC):
                        nc.tensor.matmul(
                            ps, KT[:, oc, kc * 128:(kc + 1) * 128],
                            QT[:, oc, q2 * 512:(q2 + 1) * 512],
                            start=(oc == 0), stop=(oc == OC - 1))
                    nc.scalar.activation(
                        E[:, kc, q2 * 512:(q2 + 1) * 512], ps, EXP,
                        scale=SCALE)

            if upto == 4:
                nc.sync.dma_start(out_d[0].bitcast(BF16), E[:, 0, 0:512])
                return

            for qs in range(8):
                aps0 = psum.tile([128, 512], F32, tag="ps")
                aps1 = psum.tile([128, 512], F32, tag="ps")
                dps = dpsum.tile([128, 2], F32, tag="dp")
                for kc in range(KC):
                    st = E[:, kc, qs * 128:(qs + 1) * 128]
                    nc.tensor.matmul(aps0, st, V[:, kc, 0:512],
                                     start=(kc == 0), stop=(kc == KC - 1))
                    nc.tensor.matmul(aps1, st, V[:, kc, 512:1024],
                                     start=(kc == 0), stop=(kc == KC - 1))
                    nc.tensor.matmul(dps, st, ones,
                                     start=(kc == 0), stop=(kc == KC - 1))
                rec = rpool.tile([128, 1], F32, tag="rec")
                nc.vector.reciprocal(rec, dps[:, 0:1])
                ot0 = opool.tile([128, 512], F32, tag="ot")
                nc.vector.tensor_scalar_mul(ot0, aps0, rec)
                nc.sync.dma_start(out_d[qs * 2 + 0], ot0)
                ot1 = opool.tile([128, 512], F32, tag="ot")
                nc.vector.tensor_scalar_mul(ot1, aps1, rec)
                nc.gpsimd.dma_start(out_d[qs * 2 + 1], ot1)


_NC_CACHE = {}


def build_nc(repeats=1, upto=5):
    key = (repeats, upto)
    if key not in _NC_CACHE:
        nc = bacc.Bacc("TRN2", target_bir_lowering=False, debug=False,
                       num_devices=8)
        with tile.TileContext(nc) as tc:
            io = _declare_io(nc)
            for _ in range(repeats):
                _attention_core(tc, io=io, upto=upto)
        nc.compile()
        _NC_CACHE[key] = nc
    return _NC_CACHE[key]


def _prep_xt(x2d):
    """[S, D] f32 -> [128, DC, S] bf16: t[p, dc, s] = x2d[s, dc*128+p]."""
    t = x2d.T.reshape(DC, 128, x2d.shape[0]).transpose(1, 0, 2)
    return np.ascontiguousarray(t).astype(ml_dtypes.bfloat16)


def make_in_maps(x, w):
    # w_sb[p, dc, o] = w[i][dc*128+p, o]
    w_prep = [
        np.ascontiguousarray(
            w[i].reshape(DC, 128, DO).transpose(1, 0, 2)
        ).astype(ml_dtypes.bfloat16)
        for i in range(3)
    ]
    in_maps = []
    for c in range(8):
        b, h = c // 2, c % 2
        # Bake the query half into the x layout: roll the key axis so this
        # core's query half sits at s in [0, QH).  Keys are order-invariant
        # for attention (softmax sums over all keys), so rolling the key
        # axis only permutes KT/V/E consistently and leaves out[q] exact.
        xt = _prep_xt(x[b])
        if h:
            xt = np.ascontiguousarray(np.roll(xt, -QH, axis=2))
        in_maps.append({
            "xt": xt, "wq": w_prep[0], "wk": w_prep[1], "wv": w_prep[2],
        })
    return in_maps


def assemble_out(res_list):
    out = np.empty((B, S, DO), dtype=np.float32)
    for c in range(8):
        b, h = c // 2, c % 2
        blk = res_list[c]  # [16, 128, 512]: block qs*2+oh
        core = blk.reshape(8, 2, 128, 512).transpose(0, 2, 1, 3)
        out[b, h * QH:(h + 1) * QH, :] = core.reshape(QH, DO)
    return out


def kernel(x, **rest):
    w = rest["kernel"]
    x = np.asarray(x, dtype=np.float32)
    w = np.asarray(w, dtype=np.float32)
    nc = build_nc()
    in_maps = make_in_maps(x, w)
    res = run_bass_kernel_spmd(nc, in_maps, list(range(8)))
    return assemble_out([res.results[c]["out"] for c in range(8)])
